# revision 10
# baseline (speedup 1.0000x reference)
"""Trainium2 Bass kernel for nn_Encoder segment-reduce.

Reference computation (per sample b):
    cls = onehot(argmax_k outputs[b])            # [K, HW]
    sizes = cls.sum(HW) + 0.01                   # [K]
    feat_set = feats[b] @ cls.T / sizes          # [F, K]
    out[b] = w_proj @ feat_set + bias            # [E, K]

Kernel strategy (pure data parallel: 1 sample per NeuronCore, 8 cores).

feats ship as fp8 E3M4 (4 mantissa bits) and feed the PE matmul
DIRECTLY: fp8 streams through the systolic array at bf16 speed, so no
cast pipeline exists at all, and the HBM stream drops to 8.4MB (feats)
+ 1.0MB (wT bf16) + 0.35MB (outputs f32).  E3M4 on unit gaussian data
costs rel err ~1.2e-2 vs the 2e-2 gate (e4m3 fails at 2.3e-2).

The segment-reduce matmul only uses 21 of the PE array's 128 output
columns.  The one-hot is zero-padded to 32 and consecutive hw chunks
are issued to the four 32-column array strips via tile_position
(col-tiling): strip MMs overlap, so the stream runs at the LDWEIGHTS
pace (~110ns/chunk) instead of the serial N=512 pace (216ns/chunk).
Each f-group accumulates into a [128, 512] PSUM tile (4 strips of 32
rows); four row-tiled matmuls against identity blocks sum the strips.

outputs stay f32: a bf16 argmax flips ~141/32K pixels at class-
assignment ties, and one flipped pixel shifts a whole class mean.

Loop order is fgrp-major (f-groups of 512 outer, hw chunks inner);
each f-group's strip-sum, PSUM copy, PE transposes back to f-major and
projection matmuls interleave into the next quarter's stream.

Tail: recip multiply + bias add on DVE, store pre-issued on the idle
GpSimd queue so only the transfer latency is exposed.

HAM: the PE clock ramps 1.2->2.4GHz only under ~3.4us of sustained
load.  A warmup matmul burst starting right after the engine preamble
(~5.7us) bridges the DMA ramp so the stream starts at full clock; too
many warmups delay the stream (PE queue is FIFO), so the count is
sized to end right as the first feats block lands.
"""

import numpy as np

import concourse.bacc as bacc
import concourse.bass as bass
import concourse.mybir as mybir
import concourse.tile as tile
from concourse.bass import ds, ts
from concourse.bass_utils import run_bass_kernel_spmd
from concourse.masks import make_identity

# Problem shapes (hardcoded per contract)
B = 8
K = 21
H = 64
W = 64
HW = H * W            # 4096
F = 2048
E = 256
P = 128
FC = F // P           # 16 f-chunks of 128
FG = 4                # f-groups of 512 (psum accumulate tiles)
FGW = F // FG         # 512
N_T = HW // P         # 32 hw chunks
N_CORES = 8
KP = 32               # one-hot padded to 32 classes (zeros 21..31)
NS = 4                # column strips

F32 = mybir.dt.float32
BF16 = mybir.dt.bfloat16
FP8 = mybir.dt.float8e3


def build_module(warmup=45, ns=4, store_q="gpsimd"):
    nc = bacc.Bacc("TRN2", target_bir_lowering=False, debug=False,
                   enable_partition_id=False)

    # outputs host-transposed to [p, t, k] (pixel-major).
    outputs_d = nc.dram_tensor("outputs_in", [P, N_T, K], F32, kind="ExternalInput")
    # feats [p, g, t, fj] in fp8 e3m4
    feats_d = nc.dram_tensor("feats_in", [P, FG, N_T, FGW], FP8,
                             kind="ExternalInput")
    # w_proj.T rearranged [p, fc, e]
    wT_d = nc.dram_tensor("wT_in", [P, FC, E], BF16, kind="ExternalInput")
    bias_d = nc.dram_tensor("bias_in", [1, E], F32, kind="ExternalInput")
    # out^T = [k, e] in bf16 (halves the store; host casts back to f32)
    out_d = nc.dram_tensor("out", [K, E], BF16, kind="ExternalOutput")

    with tile.TileContext(nc) as tc:
        with (
            tc.tile_pool(name="consts", bufs=1) as consts,
            tc.tile_pool(name="small", bufs=4) as small,
            tc.tile_pool(name="ps_fs", bufs=1, space="PSUM") as ps_fs,
            tc.tile_pool(name="ps_fs2", bufs=1, space="PSUM") as ps_fs2,
            tc.tile_pool(name="ps_out", bufs=1, space="PSUM") as ps_out,
        ):
            # ---- DMAs ------------------------------------------------
            # sync HWDGE queue: outputs (the onehot's prerequisite) ahead
            # of the feats stream, in consumption order.
            outputs_sb = consts.tile([P, N_T, K], F32)
            nc.sync.dma_start(out=outputs_sb[:, ds(0, 8)],
                              in_=outputs_d.ap()[:, ds(0, 8)])
            nc.sync.dma_start(out=outputs_sb[:, ds(8, 24)],
                              in_=outputs_d.ap()[:, ds(8, 24)])

            feats_sb = consts.tile([P, FG, N_T, FGW], FP8)
            FB = 8  # hw chunks per dma block

            for g in range(FG):
                for t0 in range(0, N_T, FB):
                    nc.sync.dma_start(
                        out=feats_sb[:, g, ds(t0, FB)],
                        in_=feats_d.ap()[:, g, ds(t0, FB)],
                    )

            # scalar HWDGE queue: bias + wT (wT only needed by the first
            # projection, a quarter into the stream).
            bias_sb = consts.tile([1, E], F32)
            nc.scalar.dma_start(out=bias_sb, in_=bias_d.ap())
            wT_sb = consts.tile([P, FC, E], BF16)
            nc.scalar.dma_start(out=wT_sb, in_=wT_d.ap())

            # ---- PE warm-up + constants ------------------------------
            warm_w = consts.tile([P, 64], BF16)
            nc.gpsimd.memset(warm_w, 0.0)
            # one PSUM bank shared (disjoint slices) by the projection
            # accumulator, the sizes accumulator and the warmup target.
            ps_multi = ps_out.tile([P, FGW], F32, tag="multi")
            outT_ps = ps_multi[0:KP, ds(0, E)]
            warm_ps = ps_multi[0:64, ds(320, 64)]
            for _ in range(warmup):
                nc.tensor.matmul(warm_ps, lhsT=warm_w,
                                 rhs=warm_w)

            # Preload the ACT engine's Copy activation table so the first
            # real copy doesn't eat the ~1.3us table load mid-stream.
            act_warm = small.tile([1, 1], BF16, tag="actw")
            nc.scalar.activation(out=act_warm, in_=warm_w[0:1, 0:1],
                                 func=mybir.ActivationFunctionType.Copy)

            ident = consts.tile([P, P], F32)
            make_identity(nc, ident)
            ident_b = consts.tile([P, P], BF16)
            nc.vector.tensor_copy(ident_b, ident)
            ones_f8 = consts.tile([P, 2], FP8)
            nc.vector.memset(ones_f8, 1.0)
            ones_b = consts.tile([1, KP], BF16)
            nc.vector.memset(ones_b, 1.0)
            bias_b = consts.tile([1, E], BF16)
            nc.vector.tensor_copy(bias_b, bias_sb)

            # stacked identity [32*ns, 32] for the strip sum, built from
            # same-partition copies of the identity block + one transpose.
            wideI = consts.tile([KP, KP * ns], BF16)
            for c in range(ns):
                nc.vector.tensor_copy(wideI[:, ds(KP * c, KP)],
                                      ident_b[0:KP, 0:KP])
            stackI_ps = ps_fs2.tile([KP * ns, KP], BF16, tag="fs20")
            nc.tensor.transpose(stackI_ps, wideI, ident_b[0:KP, 0:KP])
            stackI = consts.tile([KP * ns, KP], BF16)
            nc.vector.tensor_copy(stackI, stackI_ps)

            # ---- onehot (DVE; zero-padded to 32 classes) -------------
            oh_all = consts.tile([P, N_T, KP], FP8)
            nc.vector.memset(oh_all, 0.0)
            rowmax = consts.tile([P, N_T, 1], F32)

            def emit_onehot(t0, t1):
                n = t1 - t0
                nc.vector.tensor_reduce(
                    rowmax[:, ds(t0, n)], outputs_sb[:, ds(t0, n)],
                    mybir.AxisListType.X, mybir.AluOpType.max,
                )
                nc.vector.tensor_tensor(
                    oh_all[:, ds(t0, n), ds(0, K)], outputs_sb[:, ds(t0, n)],
                    rowmax[:, ds(t0, n)].to_broadcast((P, n, K)),
                    mybir.AluOpType.is_equal,
                )

            # ---- stream tiles ----------------------------------------
            fs_ps = [
                ps_fs.tile([P, FGW], F32, name=f"fs{i}", tag=f"fs{i}")
                for i in range(2)
            ]
            fs2_ps_t = ps_fs2.tile([KP, FGW], F32, name="fs2", tag="fs20")
            fs2_ps = [fs2_ps_t, fs2_ps_t]
            sz_ps = ps_fs2.tile([KP, 2], F32, tag="sz")
            fs_sbuf = [
                consts.tile([P, FGW], BF16, name=f"fsb{i}")
                for i in range(2)
            ]
            fs_sc = consts.tile([KP, F], BF16)
            fsT_sb = consts.tile([P, FC, KP], BF16)

            def emit_stream(g, t0, t1):
                for t in range(t0, t1):
                    s = t % ns
                    nc.tensor.matmul(
                        fs_ps[g % 2][ds(32 * s, 32), :],
                        lhsT=oh_all[:, t, :],
                        rhs=feats_sb[:, g, t, :],
                        start=(t < ns), stop=(t >= N_T - ns),
                        tile_position=(0, 32 * s) if ns > 1 else None,
                    )

            NSP = KP * ns  # partitions used by the strips

            # strips -> [32, 512] via 4 row-tiled matmuls against the
            # diagonal blocks of the identity (concurrent row groups).
            def emit_strip_sum(g):
                nc.scalar.activation(
                    out=fs_sbuf[g % 2][0:NSP, :], in_=fs_ps[g % 2][0:NSP, :],
                    func=mybir.ActivationFunctionType.Copy,
                )
                nc.tensor.matmul(
                    fs2_ps[g % 2],
                    lhsT=stackI[0:NSP, :],
                    rhs=fs_sbuf[g % 2][0:NSP, :],
                )

            def emit_fs2_copy(g):
                nc.vector.tensor_scalar_mul(
                    fs_sc[:, ds(g * FGW, FGW)], fs2_ps[g % 2], recip32)

            def emit_transposes(g):
                for j in range(4):
                    fc = g * 4 + j
                    trp = ps_out.tile([P, KP], BF16, name=f"trp{fc}",
                                      tag=f"trp{'AB'[fc % 2]}")
                    nc.tensor.transpose(trp, fs_sc[:, ts(fc, P)],
                                        ident_b[0:KP, 0:KP])
                    nc.scalar.activation(
                        out=fsT_sb[:, fc, :], in_=trp,
                        func=mybir.ActivationFunctionType.Copy,
                    )

            def emit_projs(g):
                for j in range(4):
                    fc = g * 4 + j
                    nc.tensor.matmul(
                        outT_ps, lhsT=fsT_sb[:, fc, :], rhs=wT_sb[:, fc, :],
                        start=(fc == 0), stop=False,
                    )

            def emit_sizes(t0, t1):
                for t in range(t0, t1):
                    nc.tensor.matmul(
                        sz_ps, lhsT=oh_all[:, t, :], rhs=ones_f8,
                        start=(t == 0), stop=(t == N_T - 1),
                    )

            def emit_fillers(n):
                for _ in range(n):
                    nc.tensor.matmul(warm_ps, lhsT=warm_w, rhs=warm_w)

            # ---- main schedule ---------------------------------------
            emit_onehot(0, 8)
            emit_onehot(8, N_T)
            emit_stream(0, 0, 8)
            emit_sizes(0, 8)
            emit_fillers(8)
            emit_stream(0, 8, 16)
            emit_sizes(8, 16)
            emit_fillers(8)
            emit_stream(0, 16, 24)
            emit_sizes(16, 24)
            emit_fillers(8)
            emit_stream(0, 24, N_T)
            emit_sizes(24, N_T)
            emit_fillers(8)

            # sizes -> recip on DVE as soon as the sizes matmuls stop
            # (emitted before the fs2 copies, which consume recip32).
            sizes_sb = small.tile([KP, 1], F32, tag="sizes")
            nc.vector.tensor_scalar_add(sizes_sb, sz_ps[:, 0:1], 0.01)
            recip32 = small.tile([KP, 1], F32, tag="recip")
            nc.vector.reciprocal(recip32, sizes_sb)

            for g in range(1, FG):
                emit_stream(g, 0, 8)
                emit_strip_sum(g - 1)
                emit_fs2_copy(g - 1)
                emit_stream(g, 8, 16)
                emit_transposes(g - 1)
                emit_stream(g, 16, 24)
                emit_projs(g - 1)
                emit_stream(g, 24, N_T)

            emit_strip_sum(FG - 1)
            emit_fs2_copy(FG - 1)
            emit_transposes(FG - 1)
            emit_projs(FG - 1)

            # ---- tail ------------------------------------------------
            # bias enters the projection accumulator as a rank-1 matmul
            # (ones[1,KP]^T @ bias[1,E]); the final copy is one ACT op.
            nc.tensor.matmul(outT_ps, lhsT=ones_b, rhs=bias_b,
                             start=False, stop=True)
            out_sb = consts.tile([K, E], BF16)
            nc.scalar.activation(out=out_sb, in_=outT_ps[0:K, :],
                                 func=mybir.ActivationFunctionType.Copy)
            # store rides the otherwise-idle gpsimd queue: the descriptor
            # is issued mid-stream and only waits on out_sb's semaphore.
            getattr(nc, store_q).dma_start(out=out_d.ap(), in_=out_sb)

    nc.compile()
    return nc


_CACHE = {}


def make_in_maps(outputs, feats, w_proj, b_proj):
    import ml_dtypes

    outputs = np.asarray(outputs, dtype=np.float32)
    # [B, K, H, W] -> per sample [p, t, k] (pixel-major: hw = t*128 + p)
    outputs_t = np.ascontiguousarray(
        outputs.reshape(B, K, N_T, P).transpose(0, 3, 2, 1)
    )
    feats = np.asarray(feats, dtype=np.float32)
    # [B, F, H, W] -> [b, p, g, t, fj] in fp8 e3m4
    feats_q = np.ascontiguousarray(
        feats.reshape(B, FG, FGW, N_T, P).transpose(0, 4, 1, 3, 2)
    ).astype(ml_dtypes.float8_e3m4)
    wT = np.ascontiguousarray(
        np.asarray(w_proj, dtype=np.float32).T
        .reshape(FC, P, E).transpose(1, 0, 2)
        .astype(ml_dtypes.bfloat16)
    )
    bias = np.ascontiguousarray(
        np.asarray(b_proj, dtype=np.float32)[None, :]
    )
    maps = []
    for b in range(B):
        maps.append({
            "outputs_in": outputs_t[b],
            "feats_in": feats_q[b],
            "wT_in": wT,
            "bias_in": bias,
        })
    return maps


def kernel(outputs, feats, w_proj, b_proj, _trace=False, _trace_kwargs=None,
           _build_kwargs=None):
    key = tuple(sorted((_build_kwargs or {}).items()))
    if key not in _CACHE:
        _CACHE[key] = build_module(**(_build_kwargs or {}))
    nc = _CACHE[key]
    in_maps = make_in_maps(outputs, feats, w_proj, b_proj)
    res = run_bass_kernel_spmd(
        nc,
        in_maps,
        core_ids=list(range(N_CORES)),
        trace=_trace,
        **(_trace_kwargs or {}),
    )
    # out is [K, E] bf16 per sample; full output is [B, E, K] f32
    out = np.stack(
        [np.asarray(r["out"]).astype(np.float32).T for r in res.results]
    )
    if _trace:
        _CACHE["last_results"] = res
    return out


# revision 12
# speedup vs baseline: 1.0285x; 1.0285x over previous
"""Trainium2 Bass kernel for nn_Encoder segment-reduce.

Reference computation (per sample b):
    cls = onehot(argmax_k outputs[b])            # [K, HW]
    sizes = cls.sum(HW) + 0.01                   # [K]
    feat_set = feats[b] @ cls.T / sizes          # [F, K]
    out[b] = w_proj @ feat_set + bias            # [E, K]

Kernel strategy (pure data parallel: 1 sample per NeuronCore, 8 cores).

feats ship as fp8 E3M4 (4 mantissa bits) and feed the PE matmul
DIRECTLY: fp8 streams through the systolic array at bf16 speed, so no
cast pipeline exists, and the HBM stream drops to 8.4MB (feats) +
1.0MB (wT bf16) + 0.35MB (outputs f32).  E3M4 on unit gaussian data
costs rel err ~1.2e-2 vs the 2e-2 gate (e4m3 fails at 2.3e-2).

The segment-reduce matmul only uses 21 of the PE array's 128 output
columns.  The one-hot is zero-padded to 32 and consecutive hw chunks
go to the four 32-column array strips via tile_position (col-tiling):
strip MMs overlap, so the stream runs near the LDWEIGHTS pace
(~110ns/chunk) instead of the serial N=512 pace (216ns/chunk).  Each
f-group accumulates into a [128, W] PSUM tile holding 4 interleaved
strip accumulators.

Per-group epilogue with NO separate strip-sum or transpose pass: for
each 128-wide f-chunk, ONE matmul lhsT=fs_strips[128,128] against
rhs=stackI[128,32] (4 stacked identities) yields the f-major,
strip-summed [128(f), 32(k)] tile in a single shot; an ACT copy and
the projection matmul against wT follow.  The per-class reciprocal is
applied during the PSUM->SBUF copy via a per-partition scalar
(recip4[128,1] = recip replicated to all 4 strips, built by one
matmul against the wide identity), so the projection accumulates
final-scale values and bias enters as a rank-1 matmul.

f-groups are UNEVEN [512,512,512,384,128]: the last group's exposed
tail epilogue is 1/4 the work of a 512-wide group.

outputs stay f32: a bf16 argmax flips ~141/32K pixels at class-
assignment ties, and one flipped pixel shifts a whole class mean.

HAM: the PE clock ramps 1.2->2.4GHz only under ~3.4us of sustained
load; dep-free filler matmuls bridge the early DMA-paced idle gaps.
"""

import numpy as np

import concourse.bacc as bacc
import concourse.bass as bass
import concourse.mybir as mybir
import concourse.tile as tile
from concourse.bass import ds, ts
from concourse.bass_utils import run_bass_kernel_spmd
from concourse.masks import make_identity

# Problem shapes (hardcoded per contract)
B = 8
K = 21
H = 64
W = 64
HW = H * W            # 4096
F = 2048
E = 256
P = 128
FC = F // P           # 16 f-chunks of 128
N_T = HW // P         # 32 hw chunks
N_CORES = 8
KP = 32               # one-hot padded to 32 classes (zeros 21..31)
NS = 4                # column strips

GW = [512, 512, 512, 384, 128]          # f-group widths
GOFF = [0, 512, 1024, 1536, 1920]       # f offsets
GFC = [w // P for w in GW]              # f-chunks per group (4,4,4,3,1)
GFC0 = [o // P for o in GOFF]           # first f-chunk per group
NG = len(GW)
# dma blocks (chunk ranges) per group
GBLK = {0: [(0, 8), (8, 16), (16, 24), (24, 32)],
        1: [(0, 8), (8, 16), (16, 24), (24, 32)],
        2: [(0, 8), (8, 16), (16, 24), (24, 32)],
        3: [(0, 8), (8, 16), (16, 24), (24, 32)],
        4: [(0, 16), (16, 28), (28, 32)]}

F32 = mybir.dt.float32
BF16 = mybir.dt.bfloat16
FP8 = mybir.dt.float8e3


def build_module(warmup=45, fillers=8, store_q="gpsimd"):
    nc = bacc.Bacc("TRN2", target_bir_lowering=False, debug=False,
                   enable_partition_id=False)

    # outputs host-transposed to [p, t, k] (pixel-major).
    outputs_d = nc.dram_tensor("outputs_in", [P, N_T, K], F32, kind="ExternalInput")
    # feats per group [p, t, w] in fp8 e3m4
    feats_d = [
        nc.dram_tensor(f"feats_{g}", [P, N_T, GW[g]], FP8, kind="ExternalInput")
        for g in range(NG)
    ]
    # w_proj.T rearranged [p, fc, e]
    wT_d = nc.dram_tensor("wT_in", [P, FC, E], BF16, kind="ExternalInput")
    bias_d = nc.dram_tensor("bias_in", [1, E], F32, kind="ExternalInput")
    # out^T = [k, e] in bf16 (halves the store; host casts back to f32)
    out_d = nc.dram_tensor("out", [K, E], BF16, kind="ExternalOutput")

    with tile.TileContext(nc) as tc:
        with (
            tc.tile_pool(name="consts", bufs=1) as consts,
            tc.tile_pool(name="small", bufs=4) as small,
            tc.tile_pool(name="ps_fs", bufs=1, space="PSUM") as ps_fs,
            tc.tile_pool(name="ps_trp", bufs=1, space="PSUM") as ps_trp,
            tc.tile_pool(name="ps_out", bufs=1, space="PSUM") as ps_out,
        ):
            # ---- DMAs ------------------------------------------------
            # sync HWDGE queue: outputs (the onehot's prerequisite) ahead
            # of the feats stream, in consumption order.
            outputs_sb = consts.tile([P, N_T, K], F32)
            nc.sync.dma_start(out=outputs_sb[:, ds(0, 8)],
                              in_=outputs_d.ap()[:, ds(0, 8)])
            nc.sync.dma_start(out=outputs_sb[:, ds(8, 24)],
                              in_=outputs_d.ap()[:, ds(8, 24)])

            feats_sb = [
                consts.tile([P, N_T, GW[g]], FP8, name=f"feats{g}")
                for g in range(NG)
            ]
            for g in range(NG):
                for (t0, t1) in GBLK[g]:
                    nc.sync.dma_start(
                        out=feats_sb[g][:, ds(t0, t1 - t0)],
                        in_=feats_d[g].ap()[:, ds(t0, t1 - t0)],
                    )

            # scalar HWDGE queue: bias + wT (wT only needed by the first
            # projection, a quarter into the stream).
            bias_sb = consts.tile([1, E], F32)
            nc.scalar.dma_start(out=bias_sb, in_=bias_d.ap())
            wT_sb = consts.tile([P, FC, E], BF16)
            nc.scalar.dma_start(out=wT_sb, in_=wT_d.ap())

            # ---- PE warm-up + constants ------------------------------
            warm_w = consts.tile([P, 64], BF16)
            nc.gpsimd.memset(warm_w, 0.0)
            ps_multi = ps_out.tile([P, 512], F32, tag="multi")
            outT_ps = ps_multi[0:KP, ds(0, E)]
            warm_ps = ps_multi[0:64, ds(320, 64)]
            for _ in range(warmup):
                nc.tensor.matmul(warm_ps, lhsT=warm_w, rhs=warm_w)

            # Preload the ACT engine's Copy activation table so the first
            # real copy doesn't eat the ~1.3us table load mid-stream.
            act_warm = small.tile([1, 1], BF16, tag="actw")
            nc.scalar.activation(out=act_warm, in_=warm_w[0:1, 0:1],
                                 func=mybir.ActivationFunctionType.Copy)

            ident = consts.tile([P, P], F32)
            make_identity(nc, ident)
            ident_b = consts.tile([P, P], BF16)
            nc.vector.tensor_copy(ident_b, ident)
            ones_f8 = consts.tile([P, 2], FP8)
            nc.vector.memset(ones_f8, 1.0)
            ones_b = consts.tile([1, KP], BF16)
            nc.vector.memset(ones_b, 1.0)
            bias_b = consts.tile([1, E], BF16)
            nc.vector.tensor_copy(bias_b, bias_sb)

            # stacked identity [128, KP] (NS stacked I32 blocks) for the
            # merge matmuls, built from same-partition copies of the
            # identity block + one PE transpose.
            wideI = consts.tile([KP, P], BF16)
            for c in range(NS):
                nc.vector.tensor_copy(wideI[:, ds(KP * c, KP)],
                                      ident_b[0:KP, 0:KP])
            stackI_ps = ps_trp.tile([P, KP], F32, name="stkps", tag="trpA")
            nc.tensor.matmul(stackI_ps, lhsT=wideI,
                             rhs=ident_b[0:KP, 0:KP])
            stackI = consts.tile([P, KP], BF16)
            nc.vector.tensor_copy(stackI, stackI_ps)

            # ---- onehot (DVE; zero-padded to 32 classes) -------------
            oh_all = consts.tile([P, N_T, KP], FP8)
            nc.vector.memset(oh_all, 0.0)
            rowmax = consts.tile([P, N_T, 1], F32)

            def emit_onehot(t0, t1):
                n = t1 - t0
                nc.vector.tensor_reduce(
                    rowmax[:, ds(t0, n)], outputs_sb[:, ds(t0, n)],
                    mybir.AxisListType.X, mybir.AluOpType.max,
                )
                nc.vector.tensor_tensor(
                    oh_all[:, ds(t0, n), ds(0, K)], outputs_sb[:, ds(t0, n)],
                    rowmax[:, ds(t0, n)].to_broadcast((P, n, K)),
                    mybir.AluOpType.is_equal,
                )

            # ---- stream tiles ----------------------------------------
            fs_ps = [
                ps_fs.tile([P, 512], F32, name=f"fs{i}", tag=f"fs{i}")
                for i in range(2)
            ]
            fs_sbuf = [
                consts.tile([P, 512], BF16, name=f"fsb{i}")
                for i in range(2)
            ]
            trpM_sb = consts.tile([P, FC, KP], BF16)
            sz_ps = ps_fs.tile([KP, 2], F32, tag="sz")

            def emit_stream(g, t0, t1):
                for t in range(t0, t1):
                    s = t % NS
                    nc.tensor.matmul(
                        fs_ps[g % 2][ds(32 * s, 32), ds(0, GW[g])],
                        lhsT=oh_all[:, t, :],
                        rhs=feats_sb[g][:, t, :],
                        start=(t < NS), stop=(t >= N_T - NS),
                        tile_position=(0, 32 * s),
                    )

            def emit_sizes(t0, t1):
                for t in range(t0, t1):
                    nc.tensor.matmul(
                        sz_ps, lhsT=oh_all[:, t, :], rhs=ones_f8,
                        start=(t == 0), stop=(t == N_T - 1),
                    )

            def emit_fillers(n):
                for _ in range(n):
                    nc.tensor.matmul(warm_ps, lhsT=warm_w, rhs=warm_w)

            # per-group epilogue: scaled PSUM->SBUF copy (DVE), then per
            # f-chunk one merge matmul (transpose+strip-sum in one shot)
            # + ACT copy + projection matmul.
            def emit_epi_copy(g):
                nc.vector.tensor_scalar_mul(
                    fs_sbuf[g % 2][:, ds(0, GW[g])],
                    fs_ps[g % 2][:, ds(0, GW[g])], recip4)

            def emit_merges(g):
                for j in range(GFC[g]):
                    fc = GFC0[g] + j
                    trp = ps_trp.tile([P, KP], F32, name=f"trp{fc}",
                                      tag=f"trp{'AB'[fc % 2]}")
                    nc.tensor.matmul(trp, lhsT=fs_sbuf[g % 2][:, ts(j, P)],
                                     rhs=stackI)
                    nc.scalar.activation(
                        out=trpM_sb[:, fc, :], in_=trp,
                        func=mybir.ActivationFunctionType.Copy,
                    )

            def emit_projs(g):
                for j in range(GFC[g]):
                    fc = GFC0[g] + j
                    nc.tensor.matmul(
                        outT_ps, lhsT=trpM_sb[:, fc, :], rhs=wT_sb[:, fc, :],
                        start=(fc == 0), stop=False,
                    )

            # ---- main schedule ---------------------------------------
            emit_onehot(0, 8)
            emit_onehot(8, N_T)
            emit_stream(0, 0, 8)
            emit_sizes(0, 8)
            emit_fillers(fillers)
            emit_stream(0, 8, 16)
            emit_sizes(8, 16)
            emit_fillers(fillers)
            emit_stream(0, 16, 24)
            emit_sizes(16, 24)
            emit_fillers(fillers)
            emit_stream(0, 24, N_T)
            emit_sizes(24, N_T)

            # sizes -> recip -> recip4 [128,1] (recip replicated to the 4
            # strips by one matmul against the wide identity).
            sizes_sb = small.tile([KP, 1], F32, tag="sizes")
            nc.vector.tensor_scalar_add(sizes_sb, sz_ps[:, 0:1], 0.01)
            recip32 = small.tile([KP, 1], F32, tag="recip")
            nc.vector.reciprocal(recip32, sizes_sb)
            recip_b = small.tile([KP, 1], BF16, tag="recipb")
            nc.vector.tensor_copy(recip_b, recip32)
            recip4_t = ps_trp.tile([P, KP], F32, name="r4ps", tag="trpB")
            emit_fillers(fillers)
            nc.tensor.matmul(recip4_t[:, 0:1], lhsT=wideI, rhs=recip_b)
            recip4 = small.tile([P, 1], F32, tag="recip4")
            nc.vector.tensor_copy(recip4, recip4_t[:, 0:1])

            for g in range(1, NG):
                emit_stream(g, 0, 8)
                emit_epi_copy(g - 1)
                emit_stream(g, 8, 16)
                emit_merges(g - 1)
                emit_stream(g, 16, 24)
                emit_projs(g - 1)
                emit_stream(g, 24, N_T)

            emit_epi_copy(NG - 1)
            emit_merges(NG - 1)
            emit_projs(NG - 1)

            # ---- tail ------------------------------------------------
            # bias enters the projection accumulator as a rank-1 matmul
            # (ones[1,KP]^T @ bias[1,E]); the final copy is one ACT op.
            nc.tensor.matmul(outT_ps, lhsT=ones_b, rhs=bias_b,
                             start=False, stop=True)
            out_sb = consts.tile([K, E], BF16)
            nc.scalar.activation(out=out_sb, in_=outT_ps[0:K, :],
                                 func=mybir.ActivationFunctionType.Copy)
            # store rides the otherwise-idle gpsimd queue: the descriptor
            # is issued mid-stream and only waits on out_sb's semaphore.
            getattr(nc, store_q).dma_start(out=out_d.ap(), in_=out_sb)

    nc.compile()
    return nc


_CACHE = {}


def make_in_maps(outputs, feats, w_proj, b_proj):
    import ml_dtypes

    outputs = np.asarray(outputs, dtype=np.float32)
    # [B, K, H, W] -> per sample [p, t, k] (pixel-major: hw = t*128 + p)
    outputs_t = np.ascontiguousarray(
        outputs.reshape(B, K, N_T, P).transpose(0, 3, 2, 1)
    )
    feats = np.asarray(feats, dtype=np.float32)
    # [B, F, H, W] -> per group [b, p, t, w] in fp8 e3m4
    f4 = feats.reshape(B, F, N_T, P)
    feats_q = {}
    for g in range(NG):
        blk = f4[:, GOFF[g]:GOFF[g] + GW[g]]        # [B, w, t, p]
        feats_q[g] = np.ascontiguousarray(
            blk.transpose(0, 3, 2, 1)
        ).astype(ml_dtypes.float8_e3m4)
    wT = np.ascontiguousarray(
        np.asarray(w_proj, dtype=np.float32).T
        .reshape(FC, P, E).transpose(1, 0, 2)
        .astype(ml_dtypes.bfloat16)
    )
    bias = np.ascontiguousarray(
        np.asarray(b_proj, dtype=np.float32)[None, :]
    )
    maps = []
    for b in range(B):
        m = {"outputs_in": outputs_t[b], "wT_in": wT, "bias_in": bias}
        for g in range(NG):
            m[f"feats_{g}"] = feats_q[g][b]
        maps.append(m)
    return maps


def kernel(outputs, feats, w_proj, b_proj, _trace=False, _trace_kwargs=None,
           _build_kwargs=None):
    key = tuple(sorted((_build_kwargs or {}).items()))
    if key not in _CACHE:
        _CACHE[key] = build_module(**(_build_kwargs or {}))
    nc = _CACHE[key]
    in_maps = make_in_maps(outputs, feats, w_proj, b_proj)
    res = run_bass_kernel_spmd(
        nc,
        in_maps,
        core_ids=list(range(N_CORES)),
        trace=_trace,
        **(_trace_kwargs or {}),
    )
    # out is [K, E] bf16 per sample; full output is [B, E, K] f32
    out = np.stack(
        [np.asarray(r["out"]).astype(np.float32).T for r in res.results]
    )
    if _trace:
        _CACHE["last_results"] = res
    return out
